# revision 18
# baseline (speedup 1.0000x reference)
"""Locoformer on 8 Trainium2 NeuronCores.

Sharding: 8-way sequence parallel. B*S = 2*2048 = 4096 tokens -> 8 chunks of
512 tokens (core c: batch c//4, seq chunk c%4). Each core runs the full
4-layer model on its 512 tokens. The sliding-window (512) attention needs a
512-token k/v halo from the left neighbor; exchanged per layer via a bf16
AllGather of (rope'd k, mixed v) with a 9-slot receive buffer (slot pid reads
rank pid-1; core 0/4's halo is garbage but masked out via key-validity bias).

Layouts: activations live feature-on-partition ("T layout", [128, chunk,
tok]); matmuls chain without transposes except q/k/o (PE transposes).
Weights host-cast to bf16, fp32 residual stream, fp32 softmax/norm stats.
"""

import sys

import numpy as np

sys.path.insert(0, "/opt/trn_rl_repo")

import ml_dtypes
import concourse.bass as bass
import concourse.mybir as mybir
import concourse.tile as tile
from concourse import bacc
from concourse.bass import ds
from concourse.bass_utils import run_bass_kernel_spmd
from concourse.masks import make_identity

F32 = mybir.dt.float32
BF16 = mybir.dt.bfloat16
F8 = mybir.dt.float8e4
DR = mybir.MatmulPerfMode.DoubleRow
AF = mybir.ActivationFunctionType

# fp8 quantization scales (powers of 2; ml_dtypes.float8_e4m3 max ~240)
SX_A = 8.0  # attn input activations (unnormalized residual x)
SW_A = 1024.0  # attn weights (std 0.02)
SX_O = 16.0  # attn gated output (pre-wo)
SX_F = 16.0  # ffn input activations (normalized)
SW1 = 1024.0  # w1 weights
SX_H = 16.0  # ffn hidden a*gelu(g)
SW2 = 1024.0  # w2 weights

B, S, DIM, H, DH, L, WIN = 2, 2048, 1024, 16, 64, 4, 512
DIN = 2730
DINP = 2816  # padded to 22*128
HC = DINP // 128  # 22 hidden chunks
FC = DIM // 128  # 8 feature chunks
TOK = 512  # tokens per core
TT = TOK // 128  # 4 token tiles
KEYS = 1024  # halo 512 + own 512
KC = KEYS // 128
EPS = 1.1920929e-07
SCALE = DH ** -0.5
NEG = -1e30
N_CORES = 8

BANDW = [128, 256, 384, 512, 512, 384, 256, 128]
BANDB = [0]
for _w in BANDW:
    BANDB.append(BANDB[-1] + _w)

KT_SZ = DIM * TOK  # kT region elems (per hp block of 128x512)
V_OFF = KT_SZ  # v region offset in kv block
KVBLK = KT_SZ + TOK * DIM  # 1 MiB elems bf16 = 2MB


def bcast_free(ap, n, pos):
    """Insert a step-0 free dim of size n at position pos (after partition)."""
    aps = [list(p) for p in ap.ap]
    aps.insert(pos, [0, n])
    return bass.AP(tensor=ap.tensor, offset=ap.offset, ap=aps)


def strided65(ap):
    """Reinterpret a [128, 1040] v_aug chunk slice as [128, 16, 64] skipping
    the ones column at 64 of each 65-block."""
    return bass.AP(
        tensor=ap.tensor, offset=ap.offset, ap=[list(ap.ap[0]), [65, 16], [1, 64]]
    )


def ones_cols(ap):
    """The 16 ones-columns (index 64 of each 65-block) of a v_aug chunk."""
    return bass.AP(
        tensor=ap.tensor, offset=ap.offset + 64, ap=[list(ap.ap[0]), [65, 16]]
    )


def eo_ap(ap, half):
    """Even/odd half-blocks of a [128, 1024] q/k tile: per head 64-col block,
    cols [0:32) (half=0) or [32:64) (half=1) -> [128, 16, 32]."""
    return bass.AP(
        tensor=ap.tensor,
        offset=ap.offset + 32 * half,
        ap=[list(ap.ap[0]), [64, 16], [1, 32]],
    )


def build_nc(single=False):
    nc = bacc.Bacc("TRN2", num_devices=1 if single else N_CORES)

    # ---- dram I/O ----
    # fp8 weights, partition-major layouts (per-partition contiguous >=512B)
    xT0 = nc.dram_tensor("xT0", [DIM, TOK], F32, kind="ExternalInput")
    wq = nc.dram_tensor("wq", [L, 2, 128, FC, 512], F8, kind="ExternalInput")
    wk = nc.dram_tensor("wk", [L, 2, 128, FC, 512], F8, kind="ExternalInput")
    wv = nc.dram_tensor("wv", [L, 2, 128, FC, 512], F8, kind="ExternalInput")
    wgm = nc.dram_tensor("wgm", [L, 128, FC, 32], F8, kind="ExternalInput")
    wo = nc.dram_tensor("wo", [L, FC, 128, FC, 128], F8, kind="ExternalInput")
    # w1 hi/lo: [L, 2*HC(hid chunk j: a at j, g at HC+j), 128, FC, 128]
    w1h = nc.dram_tensor("w1h", [L, 2 * HC, 128, FC, 128], F8, kind="ExternalInput")
    w1l = nc.dram_tensor("w1l", [L, 2 * HC, 128, FC, 128], F8, kind="ExternalInput")
    # w2 hi/lo: [L, FC(mc), 128, HC, 128]
    w2h = nc.dram_tensor("w2h", [L, FC, 128, HC, 128], F8, kind="ExternalInput")
    w2l = nc.dram_tensor("w2l", [L, FC, 128, HC, 128], F8, kind="ExternalInput")
    b1a = nc.dram_tensor("b1a", [L, DINP], F32, kind="ExternalInput")
    b1g = nc.dram_tensor("b1g", [L, DINP], F32, kind="ExternalInput")
    b2 = nc.dram_tensor("b2", [L, DIM], F32, kind="ExternalInput")
    cos_in = nc.dram_tensor("cos_in", [TOK, 32], BF16, kind="ExternalInput")
    sin_in = nc.dram_tensor("sin_in", [TOK, 32], BF16, kind="ExternalInput")
    keyvalid = nc.dram_tensor("keyvalid", [KEYS], F32, kind="ExternalInput")
    outT = nc.dram_tensor("outT", [DIM, TOK], F32, kind="ExternalOutput")

    with tile.TileContext(nc) as tc:
        import contextlib

        stack = contextlib.ExitStack()
        with stack:
            persist = stack.enter_context(tc.tile_pool(name="persist", bufs=1))
            wpool = stack.enter_context(tc.tile_pool(name="wpool", bufs=2))
            w1pool = stack.enter_context(tc.tile_pool(name="w1pool", bufs=8))
            w2pool = stack.enter_context(tc.tile_pool(name="w2pool", bufs=4))
            wopool = stack.enter_context(tc.tile_pool(name="wopool", bufs=2))
            scratch = stack.enter_context(tc.tile_pool(name="scratch", bufs=2))
            scratch2 = stack.enter_context(tc.tile_pool(name="scratch2", bufs=1))
            pbuf = stack.enter_context(tc.tile_pool(name="pbuf", bufs=2))
            small = stack.enter_context(tc.tile_pool(name="small", bufs=2))
            rowpool = stack.enter_context(tc.tile_pool(name="rowpool", bufs=1))
            dram = stack.enter_context(tc.tile_pool(name="dram", bufs=1, space="DRAM"))

            pid = nc.gpsimd.partition_id()

            # ---- persistent state ----
            xT = persist.tile([128, FC, TOK], F32)  # residual stream (T)
            x8 = persist.tile([128, FC, TOK], F8)  # fp8 mirror (x * SX_A)
            kT = persist.tile([128, FC, KEYS], BF16)  # [2-head d, keys]
            qT = persist.tile([128, FC, TOK], BF16)
            v_aug = persist.tile([128, KC, 16 * 65], BF16)  # [key, h*65]
            vres = persist.tile([128, TT, DIM], BF16)  # layer-0 v (natural)
            qkv_nat = persist.tile([128, TT, 3, DIM], BF16)  # q|k|v natural
            oT8 = persist.tile([128, FC, TOK], F8)
            hid8 = persist.tile([128, HC, TOK], F8)
            hidl = persist.tile([128, HC, TOK], F8)
            fT8 = persist.tile([128, FC, TOK], F8)
            fTl = persist.tile([128, FC, TOK], F8)
            gm_t = persist.tile([128, TT, 32], F32)  # gates | mix (natural)
            rs_q = persist.tile([128, TT, 1], F32)  # rs*scale for q
            rs_a = persist.tile([128, TT, 1], F32)  # rs for k/v/gm
            cos_t = persist.tile([128, TT, 32], BF16)
            sin_t = persist.tile([128, TT, 32], BF16)
            kv_t = persist.tile([128, KC, 1], F32)  # keyvalid bias
            m_diag = persist.tile([128, 128], F32)
            m_far = persist.tile([128, 128], F32)
            ident = persist.tile([128, 128], BF16)
            ones_bf = persist.tile([128, 1], BF16)
            one_f = persist.tile([1, 1], F32)
            rsb = persist.tile([128, TOK], F32)  # broadcast norm scale
            eps_t = persist.tile([128, 1], F32)
            lncA = persist.tile([128, 1], F32)  # ln(1/(SX_A*SW_A))
            lncF = persist.tile([1, 1], F32)  # ln(SX_F)
            b1a_all = persist.tile([128, HC], F32)
            b1g_all = persist.tile([128, HC], F32)
            b2_all = persist.tile([128, FC], F32)
            eps1 = persist.tile([1, 1], F32)

            kv_in = dram.tile([KVBLK], BF16)
            kv_out9 = dram.tile([9 * KVBLK], BF16)

            # ---- prologue ----
            for kc in range(FC):
                nc.sync.dma_start(xT[:, kc, :], xT0[128 * kc : 128 * (kc + 1), :])
                nc.scalar.activation(x8[:, kc, :], xT[:, kc, :], AF.Copy, scale=SX_A)
            for tq in range(TT):
                nc.sync.dma_start(cos_t[:, tq, :], cos_in[128 * tq : 128 * (tq + 1), :])
                nc.sync.dma_start(sin_t[:, tq, :], sin_in[128 * tq : 128 * (tq + 1), :])
            for kc in range(KC):
                nc.sync.dma_start(
                    kv_t[:, kc, :],
                    keyvalid[128 * kc : 128 * (kc + 1)].rearrange("(p o) -> p o", p=128),
                )
            nc.vector.memset(eps_t[:], EPS)
            nc.vector.memset(eps1[:], EPS)
            nc.vector.memset(ones_bf[:], 1.0)
            nc.vector.memset(one_f[:], 1.0)
            import math

            nc.vector.memset(lncA[:], math.log(1.0 / (SX_A * SW_A)))
            nc.vector.memset(lncF[:], math.log(SX_F))
            make_identity(nc, ident[:])
            # additive band masks in simT layout [key p, tok f]:
            # diag block (kc==tq): valid iff p >= f ; far block (kc==tq+4): p <= f
            nc.gpsimd.memset(m_diag[:], 0.0)
            nc.gpsimd.affine_select(
                out=m_diag[:], in_=m_diag[:], compare_op=mybir.AluOpType.is_ge,
                fill=NEG, base=0, pattern=[[-1, 128]], channel_multiplier=1,
            )
            nc.gpsimd.memset(m_far[:], 0.0)
            nc.gpsimd.affine_select(
                out=m_far[:], in_=m_far[:], compare_op=mybir.AluOpType.is_ge,
                fill=NEG, base=0, pattern=[[1, 128]], channel_multiplier=-1,
            )
            # ones columns of v_aug (persist across layers; v writes skip them)
            for kc in range(KC):
                nc.vector.memset(ones_cols(v_aug[:, kc, :]), 1.0)
            # zero slot 0 of kv_out9 so core 0's (masked) halo reads finite data
            zt = scratch2.tile([128, 1024], BF16, tag="onetime")
            nc.vector.memset(zt[:], 0.0)
            for i in range(8):
                nc.gpsimd.dma_start(
                    kv_out9[i * 131072 : (i + 1) * 131072].rearrange(
                        "(p f) -> p f", p=128
                    ),
                    zt[:],
                )

            def norm_stats(psum_pool, name):
                """sum over features of xT^2 -> psum [1, TOK] (fp32)."""
                ssq = psum_pool.tile([1, TOK], F32, tag=f"ssq{name}")
                for kc in range(FC):
                    sq = scratch.tile([128, TOK], BF16, tag="sq")
                    nc.vector.tensor_mul(sq[:], xT[:, kc, :], xT[:, kc, :])
                    nc.tensor.matmul(
                        ssq[:], ones_bf[:], sq[:],
                        start=(kc == 0), stop=(kc == FC - 1),
                    )
                ssq_sb = rowpool.tile([1, TOK], F32, tag="v1")
                nc.vector.tensor_copy(ssq_sb[:], ssq[:])
                return ssq_sb

            def rsqrt_act(dst, src_ap, eps_ap, lnv, scale=1.0, exp_bias=None):
                """dst = c*(src*scale + EPS)^-0.5 via exp(-0.5*ln(.) + ln c)."""
                nc.scalar.activation(lnv, src_ap, AF.Ln, bias=eps_ap, scale=scale)
                if exp_bias is None:
                    nc.scalar.activation(dst, lnv, AF.Exp, scale=-0.5)
                else:
                    nc.scalar.activation(dst, lnv, AF.Exp, scale=-0.5, bias=exp_bias)

            # ================= layers =================
            for l in range(L):
                # ---- attn norm scale, transposed to per-token partitions ----
                with tc.tile_pool(name=f"ps_n1_{l}", bufs=2, space="PSUM") as pp:
                    ssq_sb = norm_stats(pp, f"n1_{l}")
                    for tq in range(TT):
                        st = pp.tile([128, 1], F32, tag="stat_t")
                        nc.tensor.matmul(
                            st[:], ssq_sb[0:1, 128 * tq : 128 * (tq + 1)], one_f[:],
                            start=True, stop=True,
                        )
                        lnv128 = small.tile([128, 1], F32, tag="lnv128")
                        rsqrt_act(
                            rs_a[:, tq, :], st[:], eps_t[:], lnv128[:],
                            scale=1.0 / DIM, exp_bias=lncA[:],
                        )
                        nc.vector.tensor_scalar_mul(
                            rs_q[:, tq, :], rs_a[:, tq, :], SCALE
                        )

                # ---- projections q/k/v/gm per token tile ----
                with tc.tile_pool(name=f"ps_proj_{l}", bufs=4, space="PSUM") as pp, \
                     tc.tile_pool(name=f"ps_gm_{l}", bufs=1, space="PSUM") as ppg, \
                     tc.tile_pool(name=f"ps_tp_{l}", bufs=2, space="PSUM") as ppt:
                    # weight-type-outer streaming: alloc->use->next keeps
                    # the pool trace processable (no forward-release waits)
                    for wi, (wname, wt) in enumerate(
                        (("q", wq), ("k", wk), ("v", wv))
                    ):
                        for nb in range(2):
                            slab = wpool.tile([128, FC, 512], F8, tag="wproj")
                            nc.sync.dma_start(slab[:], wt[l, nb])
                            for tq in range(TT):
                                pt = pp.tile([128, 512], F32, tag="proj")
                                for p2 in range(FC // 2):
                                    nc.tensor.matmul(
                                        pt[:],
                                        x8[:, 2 * p2 : 2 * p2 + 2, 128 * tq : 128 * (tq + 1)],
                                        slab[:, 2 * p2 : 2 * p2 + 2, :],
                                        start=(p2 == 0), stop=(p2 == FC // 2 - 1),
                                        perf_mode=DR,
                                    )
                                rs = rs_q if wname == "q" else rs_a
                                nc.scalar.activation(
                                    qkv_nat[:, tq, wi, 512 * nb : 512 * (nb + 1)],
                                    pt[:], AF.Copy, scale=rs[:, tq, :],
                                )
                    gm_slab = wpool.tile([128, FC, 32], F8, tag="wgm")
                    nc.sync.dma_start(gm_slab[:], wgm[l])

                    for tq in range(TT):
                        qn = qkv_nat[:, tq, 0, :]
                        kn = qkv_nat[:, tq, 1, :]
                        vn = qkv_nat[:, tq, 2, :]
                        # gates/mix: sigmoid(y) = 1/(1+exp(-y))
                        pt = ppg.tile([128, 32], F32, tag="gm")
                        for p2 in range(FC // 2):
                            nc.tensor.matmul(
                                pt[:],
                                x8[:, 2 * p2 : 2 * p2 + 2, 128 * tq : 128 * (tq + 1)],
                                gm_slab[:, 2 * p2 : 2 * p2 + 2, :],
                                start=(p2 == 0), stop=(p2 == FC // 2 - 1),
                                perf_mode=DR,
                            )
                        negrs = small.tile([128, 1], F32, tag="negrs")
                        nc.vector.tensor_scalar_mul(negrs[:], rs_a[:, tq, :], -1.0)
                        eneg = small.tile([128, 32], F32, tag="eneg")
                        nc.scalar.activation(eneg[:], pt[:], AF.Exp, scale=negrs[:])
                        nc.vector.tensor_scalar_add(eneg[:], eneg[:], 1.0)
                        nc.vector.reciprocal(gm_t[:, tq, :], eneg[:])
                        # fold the o8 quant scale into the gates half
                        nc.vector.tensor_scalar_mul(
                            gm_t[:, tq, 0:16], gm_t[:, tq, 0:16], SX_O
                        )

                        # rope on q and k (E/O half-blocks, cos/sin bcast)
                        cb = bcast_free(cos_t[:, tq, :], 16, 1)
                        sb_ = bcast_free(sin_t[:, tq, :], 16, 1)
                        for t in (qn, kn):
                            tmpE = scratch.tile([128, 16, 32], BF16, tag="ropeE")
                            tmpO = scratch.tile([128, 16, 32], BF16, tag="ropeO")
                            E, O = eo_ap(t, 0), eo_ap(t, 1)
                            nc.vector.tensor_mul(tmpO[:], O, sb_)  # x_o*sin
                            nc.vector.tensor_mul(tmpE[:], E, sb_)  # x_e*sin
                            nc.vector.tensor_mul(E, E, cb)  # x_e*cos
                            nc.vector.tensor_mul(O, O, cb)  # x_o*cos
                            nc.vector.tensor_sub(E, E, tmpO[:])
                            nc.vector.tensor_add(O, O, tmpE[:])

                        # value residual lerp + write into v_aug (own keys)
                        vdst = strided65(v_aug[:, TT + tq, :])
                        if l == 0:
                            nc.vector.tensor_copy(vres[:, tq, :], vn)
                            nc.vector.tensor_copy(vdst, vn)
                        else:
                            d_ = scratch.tile([128, DIM], BF16, tag="lerp_d")
                            nc.vector.tensor_sub(d_[:], vres[:, tq, :], vn)
                            mixb = bass.AP(
                                tensor=gm_t.tensor,
                                offset=gm_t[:, tq, :].offset + 16,
                                ap=[list(gm_t[:, tq, :].ap[0]), [1, 16], [0, 64]],
                            )
                            dv = d_[:].rearrange("p (h d) -> p h d", h=16)
                            nc.vector.tensor_mul(dv, dv, mixb)
                            nc.vector.tensor_add(
                                vdst, vn.rearrange("p (h d) -> p h d", h=16), dv
                            )

                        # transpose q,k -> qT, kT(own half)
                        for hp in range(FC):
                            tp = ppt.tile([128, 128], BF16, tag="tp")
                            nc.tensor.transpose(
                                tp[:], qn[:, 128 * hp : 128 * (hp + 1)], ident[:]
                            )
                            nc.vector.tensor_copy(
                                qT[:, hp, 128 * tq : 128 * (tq + 1)], tp[:]
                            )
                            tp2 = ppt.tile([128, 128], BF16, tag="tp")
                            nc.tensor.transpose(
                                tp2[:], kn[:, 128 * hp : 128 * (hp + 1)], ident[:]
                            )
                            nc.vector.tensor_copy(
                                kT[:, hp, 512 + 128 * tq : 512 + 128 * (tq + 1)], tp2[:]
                            )

                # ---- kv exchange: send own k/v, AllGather, read halo ----
                for hp in range(FC):
                    nc.sync.dma_start(
                        kv_in[hp * 65536 : (hp + 1) * 65536].rearrange(
                            "(p f) -> p f", p=128
                        ),
                        kT[:, hp, 512:1024],
                    )
                for tq in range(TT):
                    nc.sync.dma_start(
                        kv_in[V_OFF + tq * 131072 : V_OFF + (tq + 1) * 131072].rearrange(
                            "(p h d) -> p h d", p=128, h=16
                        ),
                        strided65(v_aug[:, TT + tq, :]),
                    )
                if single:
                    # timing proxy for the AllGather: move one slot's bytes
                    nc.gpsimd.dma_start(
                        kv_out9[KVBLK : 2 * KVBLK].rearrange("(p f) -> p f", p=128),
                        kv_in[:].rearrange("(p f) -> p f", p=128),
                    )
                else:
                    nc.gpsimd.collective_compute(
                        "AllGather",
                        mybir.AluOpType.bypass,
                        replica_groups=[list(range(N_CORES))],
                        ins=[kv_in[:]],
                        outs=[kv_out9[KVBLK : 9 * KVBLK]],
                    )
                koff = pid * KVBLK
                for hp in range(FC):
                    nc.gpsimd.dma_start(
                        kT[:, hp, 0:512],
                        kv_out9[ds(koff + hp * 65536, 65536)].rearrange(
                            "(p f) -> p f", p=128
                        ),
                    )
                for kc in range(TT):
                    nc.gpsimd.dma_start(
                        strided65(v_aug[:, kc, :]),
                        kv_out9[
                            ds(koff + V_OFF + kc * 131072, 131072)
                        ].rearrange("(p h d) -> p h d", p=128, h=16),
                    )

                # ---- attention ----
                with tc.tile_pool(name=f"ps_att_{l}", bufs=3, space="PSUM") as pa, \
                     tc.tile_pool(name=f"po_att_{l}", bufs=4, space="PSUM") as po:
                    for h in range(H):
                        hp, ho = h // 2, (h % 2) * 64
                        p_sb = pbuf.tile([128, BANDB[-1]], BF16, tag="p_sb")
                        # own keys first (kc>=4) so AG latency overlaps
                        for kc in [4, 5, 6, 7, 0, 1, 2, 3]:
                            qlo = max(0, kc - 4) * 128
                            qhi = min(TT, kc + 1) * 128
                            w = qhi - qlo
                            st = pa.tile([128, 512], F32, tag="sim")
                            nc.tensor.matmul(
                                st[:, 0:w],
                                kT[ho : ho + 64, hp, 128 * kc : 128 * (kc + 1)],
                                qT[ho : ho + 64, hp, qlo:qhi],
                                start=True, stop=True,
                            )
                            if kc <= 3:  # diag sub-block tq == kc
                                off = 128 * kc - qlo
                                nc.vector.tensor_add(
                                    st[:, off : off + 128],
                                    st[:, off : off + 128],
                                    m_diag[:],
                                )
                            if kc >= 4:  # far sub-block tq == kc-4
                                off = 128 * (kc - 4) - qlo
                                nc.vector.tensor_add(
                                    st[:, off : off + 128],
                                    st[:, off : off + 128],
                                    m_far[:],
                                )
                            nc.scalar.activation(
                                p_sb[:, BANDB[kc] : BANDB[kc] + w], st[:, 0:w],
                                AF.Exp, bias=kv_t[:, kc, :],
                            )
                        for tq in range(TT):
                            ot = po.tile([128, 65], F32, tag="av")
                            for i, kc in enumerate(range(tq, tq + 5)):
                                off = BANDB[kc] + 128 * tq - max(0, kc - 4) * 128
                                nc.tensor.matmul(
                                    ot[:],
                                    p_sb[:, off : off + 128],
                                    v_aug[:, kc, 65 * h : 65 * (h + 1)],
                                    start=(i == 0), stop=(i == 4),
                                )
                            rec = small.tile([128, 1], F32, tag="rec")
                            nc.vector.reciprocal(rec[:], ot[:, 64:65])
                            nc.vector.tensor_mul(
                                rec[:], rec[:], gm_t[:, tq, h : h + 1]
                            )
                            nc.scalar.activation(
                                qkv_nat[:, tq, 0, 64 * h : 64 * (h + 1)],
                                ot[:, 0:64], AF.Copy, scale=rec[:],
                            )

                # ---- o transpose + wo + residual ----
                with tc.tile_pool(name=f"ps_wo_{l}", bufs=3, space="PSUM") as pw:
                    for tq in range(TT):
                        for hp in range(FC):
                            tp = pw.tile([128, 128], BF16, tag="tp_o")
                            nc.tensor.transpose(
                                tp[:],
                                qkv_nat[:, tq, 0, 128 * hp : 128 * (hp + 1)],
                                ident[:],
                            )
                            nc.vector.tensor_copy(
                                oT8[:, hp, 128 * tq : 128 * (tq + 1)], tp[:]
                            )
                    for mc in range(FC):
                        wos = wopool.tile([128, FC, 128], F8, tag="wo_s")
                        nc.scalar.dma_start(wos[:], wo[l, mc])
                        pr = pw.tile([128, TOK], F32, tag="wo_ps")
                        for p2 in range(FC // 2):
                            nc.tensor.matmul(
                                pr[:],
                                wos[:, 2 * p2 : 2 * p2 + 2, :],
                                oT8[:, 2 * p2 : 2 * p2 + 2, :],
                                start=(p2 == 0), stop=(p2 == FC // 2 - 1),
                                perf_mode=DR,
                            )
                        dsc = scratch2.tile([128, TOK], F32, tag="fsb")
                        nc.scalar.activation(
                            dsc[:], pr[:], AF.Copy, scale=1.0 / (SX_O * SW_A)
                        )
                        nc.vector.tensor_add(xT[:, mc, :], xT[:, mc, :], dsc[:])
                        nc.scalar.activation(
                            x8[:, mc, :], xT[:, mc, :], AF.Copy, scale=SX_A
                        )

                # ---- FFN ----
                with tc.tile_pool(name=f"ps_ffn_{l}", bufs=2, space="PSUM") as pf:
                    ssq_sb = norm_stats(pf, f"n2_{l}")
                    # combined double-rmsnorm scale on [1, TOK]:
                    # a1 = var+EPS ; t = var/a1 + EPS (=var2+EPS) ; t *= a1
                    # rs = t^-0.5   (extra +EPS inside rsqrt_act is ~6e-8 rel)
                    a1 = rowpool.tile([1, TOK], F32, tag="v2")
                    nc.vector.tensor_scalar(
                        a1[:], ssq_sb[:], 1.0 / DIM, EPS,
                        mybir.AluOpType.mult, mybir.AluOpType.add,
                    )
                    r1 = rowpool.tile([1, TOK], F32, tag="v3")
                    nc.vector.reciprocal(r1[:], a1[:])
                    nc.vector.tensor_scalar_mul(ssq_sb[:], ssq_sb[:], 1.0 / DIM)
                    nc.vector.tensor_mul(ssq_sb[:], ssq_sb[:], r1[:])
                    nc.vector.tensor_scalar_add(ssq_sb[:], ssq_sb[:], EPS)
                    nc.vector.tensor_mul(ssq_sb[:], ssq_sb[:], a1[:])
                    rsqrt_act(
                        r1[:], ssq_sb[:], eps1[:], a1[:], scale=1.0, exp_bias=lncF[:]
                    )
                    nc.gpsimd.partition_broadcast(rsb[:], r1[:])
                    for kc in range(FC):
                        # fb = SX_F * normed activations; fp8 hi + lo parts
                        fb = scratch.tile([128, TOK], BF16, tag="fbf")
                        nc.vector.tensor_mul(fb[:], xT[:, kc, :], rsb[:])
                        nc.scalar.activation(fT8[:, kc, :], fb[:], AF.Copy)
                        nc.vector.tensor_sub(fTl[:, kc, :], fb[:], fT8[:, kc, :])

                    nc.scalar.dma_start(
                        b1a_all[:], b1a[l].rearrange("(j p) -> p j", p=128)
                    )
                    nc.scalar.dma_start(
                        b1g_all[:], b1g[l].rearrange("(j p) -> p j", p=128)
                    )
                    nc.scalar.dma_start(
                        b2_all[:], b2[l].rearrange("(j p) -> p j", p=128)
                    )
                    # w1: hid[j] = gelu-gated product (hi/lo fp8 DoubleRow)
                    for j in range(HC):
                        pa_ = pf.tile([128, TOK], F32, tag="w1a")
                        pg_ = pf.tile([128, TOK], F32, tag="w1g")
                        wa_h = w1pool.tile([128, FC, 128], F8, tag="w1_s")
                        wa_l = w1pool.tile([128, FC, 128], F8, tag="w1_s")
                        wg_h = w1pool.tile([128, FC, 128], F8, tag="w1_s")
                        wg_l = w1pool.tile([128, FC, 128], F8, tag="w1_s")
                        nc.sync.dma_start(wa_h[:], w1h[l, j])
                        nc.scalar.dma_start(wa_l[:], w1l[l, j])
                        nc.sync.dma_start(wg_h[:], w1h[l, HC + j])
                        nc.scalar.dma_start(wg_l[:], w1l[l, HC + j])
                        for ps_, wh_, wl_ in ((pa_, wa_h, wa_l), (pg_, wg_h, wg_l)):
                            n_t = 3 * (FC // 2)
                            i_t = 0
                            for p2 in range(FC // 2):
                                for wslab, fslab in (
                                    (wh_, fT8), (wl_, fT8), (wh_, fTl)
                                ):
                                    nc.tensor.matmul(
                                        ps_[:],
                                        wslab[:, 2 * p2 : 2 * p2 + 2, :],
                                        fslab[:, 2 * p2 : 2 * p2 + 2, :],
                                        start=(i_t == 0), stop=(i_t == n_t - 1),
                                        perf_mode=DR,
                                    )
                                    i_t += 1
                        gsb = scratch.tile([128, TOK], BF16, tag="gsb")
                        hb = scratch.tile([128, TOK], BF16, tag="hidbf")
                        nc.scalar.activation(
                            hb[:], pa_[:], AF.Identity,
                            bias=b1a_all[:, j : j + 1], scale=SX_H / (SX_F * SW1),
                        )
                        nc.scalar.activation(
                            gsb[:], pg_[:], AF.Gelu,
                            bias=b1g_all[:, j : j + 1], scale=1.0 / (SX_F * SW1),
                        )
                        nc.vector.tensor_mul(hb[:], hb[:], gsb[:])
                        nc.scalar.activation(hid8[:, j, :], hb[:], AF.Copy)
                        nc.vector.tensor_sub(hidl[:, j, :], hb[:], hid8[:, j, :])

                    # w2 + bias + residual (hi/lo fp8 DoubleRow)
                    for mc in range(FC):
                        w2s_h = w2pool.tile([128, HC, 128], F8, tag="w2_s")
                        w2s_l = w2pool.tile([128, HC, 128], F8, tag="w2_s")
                        nc.sync.dma_start(w2s_h[:], w2h[l, mc])
                        nc.scalar.dma_start(w2s_l[:], w2l[l, mc])
                        pr = pf.tile([128, TOK], F32, tag="w2_ps")
                        n_t = 3 * (HC // 2)
                        i_t = 0
                        for p2 in range(HC // 2):
                            for wslab, hslab in (
                                (w2s_h, hid8), (w2s_l, hid8), (w2s_h, hidl)
                            ):
                                nc.tensor.matmul(
                                    pr[:],
                                    wslab[:, 2 * p2 : 2 * p2 + 2, :],
                                    hslab[:, 2 * p2 : 2 * p2 + 2, :],
                                    start=(i_t == 0), stop=(i_t == n_t - 1),
                                    perf_mode=DR,
                                )
                                i_t += 1
                        fsb = scratch2.tile([128, TOK], F32, tag="fsb")
                        nc.scalar.activation(
                            fsb[:], pr[:], AF.Identity,
                            bias=b2_all[:, mc : mc + 1], scale=1.0 / (SX_H * SW2),
                        )
                        nc.vector.tensor_add(xT[:, mc, :], xT[:, mc, :], fsb[:])
                        if l < L - 1:
                            nc.scalar.activation(
                                x8[:, mc, :], xT[:, mc, :], AF.Copy, scale=SX_A
                            )

            # ---- final rmsnorm + output ----
            with tc.tile_pool(name="ps_fin", bufs=2, space="PSUM") as pfin:
                ssq_sb = norm_stats(pfin, "fin")
                lnf = rowpool.tile([1, TOK], F32, tag="v2")
                rsf = rowpool.tile([1, TOK], F32, tag="v3")
                rsqrt_act(rsf[:], ssq_sb[:], eps1[:], lnf[:], scale=1.0 / DIM)
                nc.gpsimd.partition_broadcast(rsb[:], rsf[:])
                for kc in range(FC):
                    ot = scratch2.tile([128, TOK], F32, tag="onetime")
                    nc.vector.tensor_mul(ot[:], xT[:, kc, :], rsb[:])
                    nc.sync.dma_start(outT[128 * kc : 128 * (kc + 1), :], ot[:])

    nc.compile()
    return nc


_NC_CACHE = None
LAST_RESULT = None


def _get_nc():
    global _NC_CACHE
    if _NC_CACHE is None:
        _NC_CACHE = build_nc()
    return _NC_CACHE


def _prep_weights(inputs):
    """Host-side: permute/pad/quantize weights to fp8 layouts."""
    f8 = ml_dtypes.float8_e4m3

    def hi_lo(ws):
        hi = ws.astype(f8)
        lo = (ws - hi.astype(np.float32)).astype(f8)
        return hi, lo

    wq_ = np.asarray(inputs["wq"], np.float32)
    wkv = np.asarray(inputs["wkv"], np.float32)
    wk_, wv_ = wkv[..., : H * DH], wkv[..., H * DH :]
    # deinterleave rope pairs per head: evens then odds
    perm = np.concatenate([np.arange(0, DH, 2), np.arange(1, DH, 2)])
    full_perm = (np.arange(H)[:, None] * DH + perm[None, :]).reshape(-1)

    def quant_proj(w):  # [L, DIM, DIM] -> [L, 2, 128, FC, 512] fp8
        r = (w * SW_A).reshape(L, FC, 128, 2, 512).transpose(0, 3, 2, 1, 4)
        return np.ascontiguousarray(r).astype(f8)

    wq8 = quant_proj(wq_[:, :, full_perm])
    wk8 = quant_proj(wk_[:, :, full_perm])
    wv8 = quant_proj(wv_)
    wgm_ = np.concatenate(
        [np.asarray(inputs["wg"], np.float32), np.asarray(inputs["wmix"], np.float32)],
        axis=-1,
    )  # [L, DIM, 32]
    wgm8 = np.ascontiguousarray(
        (wgm_ * SW_A).reshape(L, FC, 128, 32).transpose(0, 2, 1, 3)
    ).astype(f8)
    wo_ = np.asarray(inputs["wo"], np.float32)  # [L, HD, DIM]
    wo8 = np.ascontiguousarray(
        (wo_ * SW_A).reshape(L, FC, 128, FC, 128).transpose(0, 3, 2, 1, 4)
    ).astype(f8)
    w1_ = np.asarray(inputs["w1"], np.float32)
    w1p = np.zeros((L, DIM, 2 * DINP), np.float32)
    w1p[:, :, :DIN] = w1_[:, :, :DIN]
    w1p[:, :, DINP : DINP + DIN] = w1_[:, :, DIN:]
    w1r = np.ascontiguousarray(
        (w1p * SW1).reshape(L, FC, 128, 2 * HC, 128).transpose(0, 3, 2, 1, 4)
    )  # [L, 2*HC, 128, FC, 128]
    w1h_, w1l_ = hi_lo(w1r)
    w2_ = np.asarray(inputs["w2"], np.float32)
    w2p = np.zeros((L, DINP, DIM), np.float32)
    w2p[:, :DIN, :] = w2_
    w2r = np.ascontiguousarray(
        (w2p * SW2).reshape(L, HC, 128, FC, 128).transpose(0, 3, 2, 1, 4)
    )  # [L, FC, 128, HC, 128]
    w2h_, w2l_ = hi_lo(w2r)
    b1_ = np.asarray(inputs["b1"], np.float32)
    b1a = np.zeros((L, DINP), np.float32)
    b1g = np.zeros((L, DINP), np.float32)
    b1a[:, :DIN] = b1_[:, :DIN] * SX_H
    b1g[:, :DIN] = b1_[:, DIN:]
    b2_ = np.asarray(inputs["b2"], np.float32)
    return dict(
        wq=wq8, wk=wk8, wv=wv8, wgm=wgm8, wo=wo8,
        w1h=w1h_, w1l=w1l_, w2h=w2h_, w2l=w2l_,
        b1a=b1a, b1g=b1g, b2=b2_,
    )


def kernel(**inputs):
    import os
    # the axon NTFF hook is absent in this container; make sure
    # run_bass_kernel_spmd never takes the trace path
    os.environ["BASS_NEVER_TRACE"] = "1"
    nc = _get_nc()
    shared = _prep_weights(inputs)
    x = np.asarray(inputs["x"], np.float32)
    inv = 1.0 / (10000.0 ** (np.arange(0, DH, 2, dtype=np.float32) / DH))
    in_maps = []
    for c in range(N_CORES):
        b, j = c // 4, c % 4
        s0 = TOK * j
        pos = (s0 + np.arange(TOK, dtype=np.float32))[:, None] * inv[None, :]
        kvv = np.zeros(KEYS, np.float32)
        if j == 0:
            kvv[:WIN] = NEG
        m = dict(shared)
        m["xT0"] = np.ascontiguousarray(x[b, s0 : s0 + TOK, :].T)
        m["cos_in"] = np.cos(pos).astype(ml_dtypes.bfloat16)
        m["sin_in"] = np.sin(pos).astype(ml_dtypes.bfloat16)
        m["keyvalid"] = kvv
        in_maps.append(m)
    global LAST_RESULT
    r = run_bass_kernel_spmd(nc, in_maps, core_ids=list(range(N_CORES)))
    LAST_RESULT = r
    out = np.zeros((B, S, DIM), np.float32)
    for c in range(N_CORES):
        b, j = c // 4, c % 4
        out[b, TOK * j : TOK * (j + 1), :] = r.results[c]["outT"].T
    return out

